# revision 45
# baseline (speedup 1.0000x reference)
"""MLA (multi-head latent attention) Trainium2 kernel, 8-core SPMD.

Strategy v2 (hardcoded for B=2, S=2048, DIM=2048, NH=16, HD=128, HDR=64,
DCKV=512, DCQ=1536):
  - Token-shard (flattened b*s, 512 tok/core) the low-rank down-projections
    (dq/dkv/kr + rope on kr), feature-major so matmuls need no transposes.
  - AllGather ONLY the small kv bundle [c_kvT | k_rT] (576 rows, bf16).
    c_q stays local: each core up-projects q for ALL 16 heads on its own 512
    tokens (only the 2432 nonzero decomposed dims), then two AllToAlls route
    q to the head owners (core c owns heads {c, c+8}); rope applied locally
    before sending.
  - Head decomposition (uniform 192 dims/head): q_h = [main 128 | ext-or-rope
    64], k_h = [main 128 | ext-or-kr 64]. Heads 0-9: main/ext from W_uq/W_uk;
    head 10: main + rope/kr; heads 11-15: rope-only with the 3 relevant W_qr
    64-row blocks PRE-SUMMED (valid since r_k is broadcast across blocks).
    Zero-padded per-core weight values keep SPMD shapes uniform; the kr
    contribution to k enters through an appended beta*I contraction block.
  - Transpose-free attention per (batch, head): scoresT [ktok x qtok], exp
    without max-subtraction, causal mask by 0/1 bf16 multiply, row-sums via
    ones-matmul, normalize after AV.
  - Out-projection computed as per-core PARTIALS (y_heads^T @ W_out slice),
    summed across cores with a ReduceScatter that also returns the output to
    token sharding. No trailing out-proj after the last collective.
  - bf16 matmul inputs (fp32 matmul is 4x slower on TRN2), fp32 PSUM.
"""
import sys

sys.path.insert(0, "/opt/trn_rl_repo")

import numpy as np
import ml_dtypes

import concourse.bass as bass
import concourse.mybir as mybir
import concourse.tile as tile
from concourse import bacc
from concourse.bass_utils import run_bass_kernel_spmd

BF = ml_dtypes.bfloat16
F32 = mybir.dt.float32
BF16 = mybir.dt.bfloat16

B, S, DIM = 2, 2048, 2048
NH, HD, HDR = 16, 128, 64
DCKV, DCQ = 512, 1536
R = 8            # cores
TL = 512         # tokens per core (flattened B*S / R)
T = B * S        # 4096
NKQ = DCQ // 128   # 12 contraction chunks for c_q
NKD = DIM // 128   # 16 for x
NKC = DCKV // 128  # 4 for c_kv
KVB = DCKV + HDR   # 576 kv-bundle rows


def _rope_rows(nc, out_ap, src_ap, cos_lo, cos_hi, sin_lo, sin_hi, tmp_pool):
    """rope on 64 feature-major rows: src/out [64, W] as two 32-row slices.
    out[0:32]  = src[0:32]*cos_lo - src[32:64]*sin_lo
    out[32:64] = src[32:64]*cos_hi + src[0:32]*sin_hi"""
    W = cos_lo.shape[-1]
    t0 = tmp_pool.tile([32, W], F32, tag="rope_t0")
    t1 = tmp_pool.tile([32, W], F32, tag="rope_t1")
    nc.vector.tensor_mul(t0[:], src_ap(0), cos_lo)
    nc.vector.tensor_mul(t1[:], src_ap(1), sin_lo)
    nc.vector.tensor_tensor(out_ap(0), t0[:], t1[:], mybir.AluOpType.subtract)
    t2 = tmp_pool.tile([32, W], F32, tag="rope_t0")
    t3 = tmp_pool.tile([32, W], F32, tag="rope_t1")
    nc.vector.tensor_mul(t2[:], src_ap(1), cos_hi)
    nc.vector.tensor_mul(t3[:], src_ap(0), sin_hi)
    nc.vector.tensor_tensor(out_ap(1), t2[:], t3[:], mybir.AluOpType.add)


def build_nc(reps=1):
    nc = bacc.Bacc(None, target_bir_lowering=False, debug=False)
    dt_in = {}

    def din(name, shape, dt=BF16):
        t = nc.dram_tensor(name, list(shape), dt, kind="ExternalInput")
        dt_in[name] = t
        return t

    din("xT", (DIM, TL))
    din("cosT_c", (HDR, TL))
    din("sinT_c", (HDR, TL))
    din("WdqT", (DIM, DCQ))
    din("WdkvT", (DIM, DCKV))
    din("WkrT", (DIM, HDR))
    din("WqAT", (DCQ, 2048))    # plain q blocks (replicated), see _prep_inputs
    din("WqRT", (DCQ, 384))     # rope q blocks h10..15 (pre-summed, scaled)
    din("Wk1T", (DCKV, 256))    # k main per slot (per-core)
    din("Wk2aT", (DCKV, 128))   # k ext per slot (per-core)
    din("Wk2bT", (HDR, 128))    # beta * I64 per slot (kr contribution)
    din("WvT", (DCKV, 256))     # v per slot
    din("WoT", (256, DIM))      # W_out cols for my 2 heads, transposed
    outT = nc.dram_tensor("outT", [TL, DIM], BF16, kind="ExternalOutput")

    with tile.TileContext(nc) as tc:
        with tc.tile_pool(name="const", bufs=1) as const, \
             tc.tile_pool(name="dram", bufs=1, space="DRAM") as dram:
            ones = const.tile([128, 1], BF16, tag="ones")
            nc.gpsimd.memset(ones[:], 1.0)
            zeros = const.tile([128, TL], BF16, tag="zeros")
            nc.gpsimd.memset(zeros[:], 0.0)
            masks = []
            for s in range(4):  # keep iff q >= k : y >= p + s*128
                m = const.tile([128, TL], BF16, tag=f"mask{s}")
                nc.gpsimd.memset(m[:], 1.0)
                nc.gpsimd.affine_select(out=m[:], in_=m[:],
                                        compare_op=mybir.AluOpType.is_ge, fill=0.0,
                                        base=-s * 128, pattern=[[1, TL]],
                                        channel_multiplier=-1)
                masks.append(m)

            for _rep in range(reps):
                sfx = f"_{_rep}"
                bounce = dram.tile([KVB, TL], BF16, tag="bounce" + sfx,
                                   name="bounce" + sfx)
                gath = dram.tile([R, KVB, TL], BF16, tag="gath" + sfx,
                                 name="gath" + sfx, addr_space="Shared")
                a2a_in = [dram.tile([R, 192, TL], BF16, tag=f"a2ain{s}{sfx}",
                                    name=f"a2ain{s}{sfx}") for s in range(2)]
                a2a_out = [dram.tile([R, 192, TL], BF16, tag=f"a2aout{s}{sfx}",
                                     name=f"a2aout{s}{sfx}") for s in range(2)]
                rs_in = dram.tile([R, TL, DIM], BF16, tag="rsin" + sfx,
                                  name="rsin" + sfx)
                rs_out = dram.tile([TL, DIM], BF16, tag="rsout" + sfx,
                                   name="rsout" + sfx)
                _phase(nc, tc, ones, zeros, masks,
                       bounce, gath, a2a_in, a2a_out, rs_in, rs_out,
                       dt_in, outT, sfx)

    nc.compile()
    return nc


def _phase(nc, tc, ones, zeros, masks,
           bounce, gath, a2a_in, a2a_out, rs_in, rs_out, dt_in, outT, sfx):
    # ================= Phase A: local down-projections =================
    # cqsb (local c_q, bf16) survives into phase Q.
    with tc.tile_pool(name="pcq" + sfx, bufs=1) as pcq:
        cqsb = pcq.tile([128, NKQ, TL], BF16, tag="cqsb")
        wqa = pcq.tile([128, NKQ, 2048], BF16, tag="wqa")
        wqr = pcq.tile([128, NKQ, 384], BF16, tag="wqr")
        csb = pcq.tile([HDR, TL], BF16, tag="csb")
        ssb = pcq.tile([HDR, TL], BF16, tag="ssb")
        with tc.tile_pool(name="paw" + sfx, bufs=1) as paw, \
             tc.tile_pool(name="pas" + sfx, bufs=3) as pas, \
             tc.tile_pool(name="paps" + sfx, bufs=2, space="PSUM") as paps:
            xsb = paw.tile([128, NKD, TL], BF16, tag="xsb")
            wdkv = paw.tile([128, NKD, DCKV], BF16, tag="wdkv")
            xr = dt_in["xT"].rearrange("(ko p) t -> p ko t", p=128)
            wr = dt_in["WdkvT"].rearrange("(ko p) n -> p ko n", p=128)
            for kc in range(4):
                ksl = slice(4 * kc, 4 * kc + 4)
                nc.sync.dma_start(xsb[:, ksl, :], xr[:, ksl, :])
                nc.sync.dma_start(wdkv[:, ksl, :], wr[:, ksl, :])
            wkr = paw.tile([128, NKD, HDR], BF16, tag="wkr")
            nc.sync.dma_start(wkr[:], dt_in["WkrT"].rearrange("(ko p) n -> p ko n", p=128))
            nc.sync.dma_start(csb[:], dt_in["cosT_c"][:])
            nc.sync.dma_start(ssb[:], dt_in["sinT_c"][:])
            wdq = paw.tile([128, NKD, DCQ], BF16, tag="wdq")
            dq_r = dt_in["WdqT"].rearrange("(ko p) n -> p ko n", p=128)
            with tc.tile_wait_until(0.022):
                for kc in range(4):
                    ksl = slice(4 * kc, 4 * kc + 4)
                    nc.sync.dma_start(wdq[:, ksl, :], dq_r[:, ksl, :])
            nc.sync.dma_start(wqa[:], dt_in["WqAT"].rearrange("(ko p) n -> p ko n", p=128))
            nc.sync.dma_start(wqr[:], dt_in["WqRT"].rearrange("(ko p) n -> p ko n", p=128))

            # c_kv -> bounce rows [0, 512)
            for m in range(NKC):
                ps = paps.tile([128, TL], F32, tag="psA", name="psA")
                for k in range(NKD):
                    nc.tensor.matmul(ps[:], wdkv[:, k, m * 128:(m + 1) * 128],
                                     xsb[:, k, :], start=(k == 0), stop=(k == NKD - 1))
                ev = pas.tile([128, TL], BF16, tag="evA", name="evA")
                nc.scalar.copy(ev[:], ps[:])
                nc.sync.dma_start(bounce[m * 128:(m + 1) * 128, :], ev[:])
            # roped k_r -> bounce rows [512, 576)
            ps = paps.tile([64, TL], F32, tag="pskr")
            for k in range(NKD):
                nc.tensor.matmul(ps[:], wkr[:, k, :], xsb[:, k, :],
                                 start=(k == 0), stop=(k == NKD - 1))
            krr = pas.tile([64, TL], BF16, tag="krr")
            _rope_rows(nc,
                       lambda i: krr[i * 32:(i + 1) * 32, :],
                       lambda i: ps[i * 32:(i + 1) * 32, :],
                       csb[0:32, :], csb[32:64, :], ssb[0:32, :], ssb[32:64, :], pas)
            nc.gpsimd.dma_start(bounce[DCKV:KVB, :], krr[:])
            # c_q -> cqsb (stays on-core)
            for m in range(NKQ):
                ps = paps.tile([128, TL], F32, tag="psA", name="psA")
                for k in range(NKD):
                    nc.tensor.matmul(ps[:], wdq[:, k, m * 128:(m + 1) * 128],
                                     xsb[:, k, :], start=(k == 0), stop=(k == NKD - 1))
                nc.scalar.copy(cqsb[:, m, :], ps[:])

        nc.gpsimd.collective_compute(
            "AllGather", mybir.AluOpType.bypass,
            replica_groups=[list(range(R))],
            ins=[bounce.opt()], outs=[gath.opt()])

        # ============ Phase Q: local q up-projection for all 16 heads ============
        # WqAT col layout (plain blocks, already scaled):
        #   ob 0..7  : slot0 main of head d=ob        -> a2a_in[0][d, 0:128]
        #   ob 8..11 : slot0 ext, heads 2(ob-8)+{0,1} -> a2a_in[0][d, 128:192]
        #   ob 12..14: slot1 main of head 8+(ob-12)   -> a2a_in[1][ob-12, 0:128]
        #   ob 15    : slot1 ext h8 (rows 0:64), h9 (rows 64:128)
        # WqRT: rope blocks rb=0..2 hold heads {10+2rb, 11+2rb} (64 rows each).
        with tc.tile_pool(name="pqs" + sfx, bufs=3) as pqs, \
             tc.tile_pool(name="pqps" + sfx, bufs=2, space="PSUM") as pqps:
            # zero-fill slot1 mains for heads 11..15 (chunks d=3..7)
            for d2 in range(3, 8):
                nc.sync.dma_start(a2a_in[1][d2, 0:128, :], zeros[:])

            for ob in range(16):
                ps = pqps.tile([128, TL], F32, tag="psQ", name="psQ")
                for k in range(NKQ):
                    nc.tensor.matmul(ps[:], wqa[:, k, ob * 128:(ob + 1) * 128],
                                     cqsb[:, k, :], start=(k == 0), stop=(k == NKQ - 1))
                ev = pqs.tile([128, TL], BF16, tag="evQ", name="evQ")
                nc.scalar.copy(ev[:], ps[:])
                if ob < 8:
                    nc.sync.dma_start(a2a_in[0][ob, 0:128, :], ev[:])
                elif ob < 12:
                    for half in range(2):
                        d = 2 * (ob - 8) + half
                        nc.sync.dma_start(a2a_in[0][d, 128:192, :],
                                          ev[half * 64:(half + 1) * 64, :])
                elif ob < 15:
                    nc.sync.dma_start(a2a_in[1][ob - 12, 0:128, :], ev[:])
                else:
                    for half in range(2):
                        nc.sync.dma_start(a2a_in[1][half, 128:192, :],
                                          ev[half * 64:(half + 1) * 64, :])
            for rb in range(3):
                ps = pqps.tile([128, TL], F32, tag="psQ", name="psQ")
                for k in range(NKQ):
                    nc.tensor.matmul(ps[:], wqr[:, k, rb * 128:(rb + 1) * 128],
                                     cqsb[:, k, :], start=(k == 0), stop=(k == NKQ - 1))
                for half in range(2):
                    h = 10 + 2 * rb + half
                    rr = pqs.tile([64, TL], BF16, tag="evQ", name="evQ")
                    off = half * 64
                    _rope_rows(nc,
                               lambda i: rr[i * 32:(i + 1) * 32, :],
                               lambda i, off=off: ps[off + i * 32:off + (i + 1) * 32, :],
                               csb[0:32, :], csb[32:64, :],
                               ssb[0:32, :], ssb[32:64, :], pqs)
                    nc.sync.dma_start(a2a_in[1][h - 8, 128:192, :], rr[:])

    nc.gpsimd.collective_compute(
        "AllToAll", mybir.AluOpType.bypass,
        replica_groups=[list(range(R))],
        ins=[a2a_in[0].opt()], outs=[a2a_out[0].opt()])
    nc.gpsimd.collective_compute(
        "AllToAll", mybir.AluOpType.bypass,
        replica_groups=[list(range(R))],
        ins=[a2a_in[1].opt()], outs=[a2a_out[1].opt()])

    # ============ Phase B: k/v up-projection + attention + out partials ============
    # tile_wait_until: keep the scheduler from hoisting collective-dependent
    # phase-B work ahead of the local q path in the in-order engine queues.
    with tc.tile_wait_until(0.2), \
         tc.tile_pool(name="pbw" + sfx, bufs=1) as pbw, \
         tc.tile_pool(name="pbig" + sfx, bufs=1) as pbig:
        wo = pbw.tile([128, 2, DIM], BF16, tag="wo")
        nc.sync.dma_start(wo[:], dt_in["WoT"].rearrange("(s p) n -> p s n", p=128))

        K1 = pbig.tile([128, 2, R, TL], BF16, tag="K1", name="K1")
        K2 = pbig.tile([128, 2, R, TL], BF16, tag="K2", name="K2")
        V = pbig.tile([128, 32, 256], BF16, tag="V", name="V")
        Q1 = [pbig.tile([128, R, TL], BF16, tag=f"Q1_{s}", name=f"Q1_{s}")
              for s in range(2)]
        Q2 = [pbig.tile([128, R, TL], BF16, tag=f"Q2_{s}", name=f"Q2_{s}")
              for s in range(2)]
        Y = pbig.tile([128, 2, B, 4, TL], BF16, tag="Y", name="Y")

        def warm(pool, tag, n, cols=512):
            for _ in range(n):
                wps = pool.tile([128, cols], F32, tag=tag, name=tag)
                nc.tensor.matmul(wps[:], zeros[:, 0:128], zeros[:], start=True, stop=True)

        with tc.tile_pool(name="pkw" + sfx, bufs=1) as pkw, \
             tc.tile_pool(name="pbc" + sfx, bufs=3) as pbc, \
             tc.tile_pool(name="pbps" + sfx, bufs=2, space="PSUM") as pbps, \
             tc.tile_pool(name="pbp2" + sfx, bufs=2, space="PSUM") as pbp2:
            warm(pbps, "psb", 12)
            wk1 = pkw.tile([128, NKC, 256], BF16, tag="wk1")
            nc.sync.dma_start(wk1[:], dt_in["Wk1T"].rearrange("(ko p) n -> p ko n", p=128))
            wk2a = pkw.tile([128, NKC, 128], BF16, tag="wk2a")
            nc.sync.dma_start(wk2a[:], dt_in["Wk2aT"].rearrange("(ko p) n -> p ko n", p=128))
            wk2b = pkw.tile([128, 128], BF16, tag="wk2b")
            nc.sync.dma_start(wk2b[0:64, :], dt_in["Wk2bT"][:])
            wv = pkw.tile([128, NKC, 256], BF16, tag="wv")
            nc.sync.dma_start(wv[:], dt_in["WvT"].rearrange("(ko p) n -> p ko n", p=128))

            for rt in range(R):
                ckv = pbc.tile([128, NKC, TL], BF16, tag="ckvcol", name="ckvcol")
                nc.gpsimd.dma_start(ckv[:], gath[rt, 0:DCKV, :]
                                    .rearrange("(ko p) t -> p ko t", p=128))
                krg = pbc.tile([128, TL], BF16, tag="krg", name="krg")
                nc.gpsimd.dma_start(krg[0:64, :], gath[rt, DCKV:KVB, :])
                for s in range(2):
                    ps = pbps.tile([128, TL], F32, tag="psb", name="psb")
                    for k in range(NKC):
                        nc.tensor.matmul(ps[:], wk1[:, k, s * 128:(s + 1) * 128],
                                         ckv[:, k, :], start=(k == 0), stop=(k == NKC - 1))
                    nc.scalar.copy(K1[:, s, rt, :], ps[:])
                ps = pbps.tile([128, TL], F32, tag="psb", name="psb")
                for k in range(NKC):
                    nc.tensor.matmul(ps[:], wk2a[:, k, :], ckv[:, k, :],
                                     start=(k == 0), stop=False)
                nc.tensor.matmul(ps[:], wk2b[0:64, :], krg[0:64, :], start=False, stop=True)
                for s in range(2):
                    nc.scalar.copy(K2[0:64, s, rt, :], ps[s * 64:(s + 1) * 64, :])
                for js in range(4):
                    ps = pbp2.tile([128, 256], F32, tag="psv", name="psv")
                    for k in range(NKC):
                        nc.tensor.matmul(ps[:], ckv[:, k, js * 128:(js + 1) * 128],
                                         wv[:, k, :], start=(k == 0), stop=(k == NKC - 1))
                    nc.scalar.copy(V[:, rt * 4 + js, :], ps[:])

        with tc.tile_pool(name="pat" + sfx, bufs=8) as pat, \
             tc.tile_pool(name="pan" + sfx, bufs=4) as pan, \
             tc.tile_pool(name="psS" + sfx, bufs=3, space="PSUM") as psS, \
             tc.tile_pool(name="psY" + sfx, bufs=2, space="PSUM") as psY, \
             tc.tile_pool(name="psL" + sfx, bufs=1, space="PSUM") as psL, \
             tc.tile_pool(name="pcps" + sfx, bufs=2, space="PSUM") as pcps, \
             tc.tile_pool(name="pce" + sfx, bufs=6) as pce:
            for s in range(2):
                with tc.tile_wait_until(0.3 if s == 0 else 0.42):
                    for src in range(R):
                        nc.gpsimd.dma_start(Q1[s][:, src, :], a2a_out[s][src, 0:128, :])
                        nc.gpsimd.dma_start(Q2[s][0:64, src, :], a2a_out[s][src, 128:192, :])

        def do_attn(b, s):
            for it in range(4):
                rti = b * 4 + it
                nj = 4 * (it + 1)
                py = psY.tile([128, TL], F32, tag="py", name="py")
                pl = psL.tile([1, TL], F32, tag="pl", name="pl")
                for j in range(nj):
                    rtj = b * 4 + j // 4
                    sub = j % 4
                    sl = slice(sub * 128, (sub + 1) * 128)
                    pss = psS.tile([128, TL], F32, tag="pss", name="pss")
                    nc.tensor.matmul(pss[:], K1[:, s, rtj, sl], Q1[s][:, rti, :],
                                     start=True, stop=False)
                    nc.tensor.matmul(pss[:], K2[0:64, s, rtj, sl], Q2[s][0:64, rti, :],
                                     start=False, stop=True)
                    et = pat.tile([128, TL], BF16, tag="et", name="et")
                    d = j - 4 * it
                    if d >= 0:
                        er = pat.tile([128, TL], BF16, tag="er", name="er")
                        nc.scalar.activation(er[:], pss[:],
                                             mybir.ActivationFunctionType.Exp)
                        nc.vector.tensor_mul(et[:], er[:], masks[d][:])
                    else:
                        nc.scalar.activation(et[:], pss[:],
                                             mybir.ActivationFunctionType.Exp)
                    jj = b * 16 + j
                    nc.tensor.matmul(py[:], V[:, jj, s * 128:(s + 1) * 128], et[:],
                                     start=(j == 0), stop=(j == nj - 1))
                    nc.tensor.matmul(pl[:], ones[:], et[:],
                                     start=(j == 0), stop=(j == nj - 1))
                rec = pan.tile([1, TL], F32, tag="rec", name="rec")
                nc.vector.reciprocal(rec[:], pl[:])
                rb = pan.tile([128, TL], F32, tag="rb", name="rb")
                nc.gpsimd.partition_broadcast(rb[:], rec[:])
                nc.vector.tensor_mul(Y[:, s, b, it, :], py[:], rb[:])

        def do_out(b):
            # rs_in[b*4+it][tok 512, DIM] partials: out[t, o] = sum_s Y_s^T Wo_s
            for it in range(4):
                for sub in range(4):
                    tsl = slice(sub * 128, (sub + 1) * 128)
                    for obk in range(4):
                        osl = slice(obk * 512, (obk + 1) * 512)
                        po = pcps.tile([128, 512], F32, tag="po", name="po")
                        nc.tensor.matmul(po[:], Y[:, 0, b, it, tsl], wo[:, 0, osl],
                                         start=True, stop=False)
                        nc.tensor.matmul(po[:], Y[:, 1, b, it, tsl], wo[:, 1, osl],
                                         start=False, stop=True)
                        ev = pce.tile([128, 512], BF16, tag="evO", name="evO")
                        if obk % 2 == 0:
                            nc.scalar.copy(ev[:], po[:])
                        else:
                            nc.vector.tensor_copy(ev[:], po[:])
                        nc.sync.dma_start(rs_in[b * 4 + it, tsl, osl], ev[:])

        do_attn(0, 0)
        do_attn(1, 0)
        do_attn(0, 1)
        do_out(0)
        do_attn(1, 1)
        do_out(1)

    nc.gpsimd.collective_compute(
        "ReduceScatter", mybir.AluOpType.add,
        replica_groups=[list(range(R))],
        ins=[rs_in.opt()], outs=[rs_out.opt()])
    nc.sync.dma_start(outT[:], rs_out[:])


def _prep_inputs(inputs):
    """Host-side sharding: returns in_maps list of 8 dicts."""
    x = np.asarray(inputs["x"], np.float32)
    cos = np.asarray(inputs["freq_cos"], np.float32)
    sin = np.asarray(inputs["freq_sin"], np.float32)
    for bn in ("b_dq", "b_uq", "b_qr", "b_dkv", "b_uk", "b_uv", "b_kr", "b_out"):
        assert np.abs(np.asarray(inputs[bn])).max() == 0.0, f"{bn} nonzero"
    W_dq = np.asarray(inputs["W_dq"], np.float32)
    W_uq = np.asarray(inputs["W_uq"], np.float32)
    W_qr = np.asarray(inputs["W_qr"], np.float32)
    W_dkv = np.asarray(inputs["W_dkv"], np.float32)
    W_uk = np.asarray(inputs["W_uk"], np.float32)
    W_uv = np.asarray(inputs["W_uv"], np.float32)
    W_kr = np.asarray(inputs["W_kr"], np.float32)
    W_out = np.asarray(inputs["W_out"], np.float32)

    scale = 1.0 / np.float32(np.sqrt(HD + HDR))
    xf = x.reshape(T, DIM)
    cosT = np.ascontiguousarray(cos.T).astype(BF)   # [64, 2048]
    sinT = np.ascontiguousarray(sin.T).astype(BF)
    WdqT = np.ascontiguousarray(W_dq.T).astype(BF)
    WdkvT = np.ascontiguousarray(W_dkv.T).astype(BF)
    WkrT = np.ascontiguousarray(W_kr.T).astype(BF)

    # replicated q weights: plain blocks [2048, DCQ] + rope blocks [384, DCQ]
    WqA = np.empty((2048, DCQ), np.float32)
    for h in range(8):
        WqA[128 * h:128 * (h + 1)] = W_uq[192 * h:192 * h + 128]
        WqA[1024 + 64 * h:1024 + 64 * (h + 1)] = W_uq[192 * h + 128:192 * h + 192]
    WqA[1536:1664] = W_uq[1536:1664]   # h8 main
    WqA[1664:1792] = W_uq[1728:1856]   # h9 main
    WqA[1792:1920] = W_uq[1920:2048]   # h10 main
    WqA[1920:1984] = W_uq[1664:1728]   # h8 ext
    WqA[1984:2048] = W_uq[1856:1920]   # h9 ext
    WqA *= scale
    WqR = np.empty((384, DCQ), np.float32)
    WqR[0:64] = W_qr[0:64]             # h10 rope
    for h in range(11, 16):
        r0 = 192 * h - 2048
        WqR[64 * (h - 10):64 * (h - 9)] = (W_qr[r0:r0 + 64]
                                           + W_qr[r0 + 64:r0 + 128]
                                           + W_qr[r0 + 128:r0 + 192])
    WqR *= scale
    WqAT = np.ascontiguousarray(WqA.T).astype(BF)
    WqRT = np.ascontiguousarray(WqR.T).astype(BF)

    def k_parts(h):
        km = np.zeros((128, DCKV), np.float32)
        ke = np.zeros((64, DCKV), np.float32)
        beta = 0.0
        if h <= 9:
            km[:] = W_uk[192 * h:192 * h + 128]
            ke[:] = W_uk[192 * h + 128:192 * h + 192]
        elif h == 10:
            km[:] = W_uk[1920:2048]
            beta = 1.0
        else:
            beta = 1.0
        return km, ke, beta

    in_maps = []
    for c in range(R):
        heads = (c, c + 8)
        Wk1 = np.zeros((256, DCKV), np.float32)
        Wk2a = np.zeros((128, DCKV), np.float32)
        Wk2b = np.zeros((128, HDR), np.float32)
        Wv = np.zeros((256, DCKV), np.float32)
        for s, h in enumerate(heads):
            km, ke, beta = k_parts(h)
            Wk1[128 * s:128 * (s + 1)] = km
            Wk2a[64 * s:64 * (s + 1)] = ke
            Wk2b[64 * s:64 * (s + 1)] = beta * np.eye(HDR, dtype=np.float32)
            Wv[128 * s:128 * (s + 1)] = W_uv[128 * h:128 * (h + 1)]
        WoT = np.concatenate(
            [np.ascontiguousarray(W_out[:, 128 * h:128 * (h + 1)].T) for h in heads],
            axis=0)
        p0 = (c % 4) * TL
        in_maps.append({
            "xT": np.ascontiguousarray(xf[c * TL:(c + 1) * TL].T).astype(BF),
            "cosT_c": np.ascontiguousarray(cosT[:, p0:p0 + TL]),
            "sinT_c": np.ascontiguousarray(sinT[:, p0:p0 + TL]),
            "WdqT": WdqT, "WdkvT": WdkvT, "WkrT": WkrT,
            "WqAT": WqAT, "WqRT": WqRT,
            "Wk1T": np.ascontiguousarray(Wk1.T).astype(BF),
            "Wk2aT": np.ascontiguousarray(Wk2a.T).astype(BF),
            "Wk2bT": np.ascontiguousarray(Wk2b.T).astype(BF),
            "WvT": np.ascontiguousarray(Wv.T).astype(BF),
            "WoT": WoT.astype(BF),
        })
    return in_maps


_NC_CACHE = {}


def get_nc(reps=1):
    if reps not in _NC_CACHE:
        _NC_CACHE[reps] = build_nc(reps)
    return _NC_CACHE[reps]


def kernel(**inputs) -> np.ndarray:
    nc = get_nc()
    in_maps = _prep_inputs(inputs)
    res = run_bass_kernel_spmd(nc, in_maps, core_ids=list(range(R)))
    out = np.empty((T, DIM), np.float32)
    for c in range(R):
        out[c * TL:(c + 1) * TL] = res.results[c]["outT"].astype(np.float32)
    return out.reshape(B, S, DIM)


# revision 46
# speedup vs baseline: 1.0022x; 1.0022x over previous
"""MLA (multi-head latent attention) Trainium2 kernel, 8-core SPMD.

Strategy v2 (hardcoded for B=2, S=2048, DIM=2048, NH=16, HD=128, HDR=64,
DCKV=512, DCQ=1536):
  - Token-shard (flattened b*s, 512 tok/core) the low-rank down-projections
    (dq/dkv/kr + rope on kr), feature-major so matmuls need no transposes.
  - AllGather ONLY the small kv bundle [c_kvT | k_rT] (576 rows, bf16).
    c_q stays local: each core up-projects q for ALL 16 heads on its own 512
    tokens (only the 2432 nonzero decomposed dims), then two AllToAlls route
    q to the head owners (core c owns heads {c, c+8}); rope applied locally
    before sending.
  - Head decomposition (uniform 192 dims/head): q_h = [main 128 | ext-or-rope
    64], k_h = [main 128 | ext-or-kr 64]. Heads 0-9: main/ext from W_uq/W_uk;
    head 10: main + rope/kr; heads 11-15: rope-only with the 3 relevant W_qr
    64-row blocks PRE-SUMMED (valid since r_k is broadcast across blocks).
    Zero-padded per-core weight values keep SPMD shapes uniform; the kr
    contribution to k enters through an appended beta*I contraction block.
  - Transpose-free attention per (batch, head): scoresT [ktok x qtok], exp
    without max-subtraction, causal mask by 0/1 bf16 multiply, row-sums via
    ones-matmul, normalize after AV.
  - Out-projection computed as per-core PARTIALS (y_heads^T @ W_out slice),
    summed across cores with a ReduceScatter that also returns the output to
    token sharding. No trailing out-proj after the last collective.
  - bf16 matmul inputs (fp32 matmul is 4x slower on TRN2), fp32 PSUM.
"""
import sys

sys.path.insert(0, "/opt/trn_rl_repo")

import numpy as np
import ml_dtypes

import concourse.bass as bass
import concourse.mybir as mybir
import concourse.tile as tile
from concourse import bacc
from concourse.bass_utils import run_bass_kernel_spmd

BF = ml_dtypes.bfloat16
F32 = mybir.dt.float32
BF16 = mybir.dt.bfloat16

B, S, DIM = 2, 2048, 2048
NH, HD, HDR = 16, 128, 64
DCKV, DCQ = 512, 1536
R = 8            # cores
TL = 512         # tokens per core (flattened B*S / R)
T = B * S        # 4096
NKQ = DCQ // 128   # 12 contraction chunks for c_q
NKD = DIM // 128   # 16 for x
NKC = DCKV // 128  # 4 for c_kv
KVB = DCKV + HDR   # 576 kv-bundle rows


def _rope_rows(nc, out_ap, src_ap, cos_lo, cos_hi, sin_lo, sin_hi, tmp_pool):
    """rope on 64 feature-major rows: src/out [64, W] as two 32-row slices.
    out[0:32]  = src[0:32]*cos_lo - src[32:64]*sin_lo
    out[32:64] = src[32:64]*cos_hi + src[0:32]*sin_hi"""
    W = cos_lo.shape[-1]
    t0 = tmp_pool.tile([32, W], F32, tag="rope_t0")
    t1 = tmp_pool.tile([32, W], F32, tag="rope_t1")
    nc.vector.tensor_mul(t0[:], src_ap(0), cos_lo)
    nc.vector.tensor_mul(t1[:], src_ap(1), sin_lo)
    nc.vector.tensor_tensor(out_ap(0), t0[:], t1[:], mybir.AluOpType.subtract)
    t2 = tmp_pool.tile([32, W], F32, tag="rope_t0")
    t3 = tmp_pool.tile([32, W], F32, tag="rope_t1")
    nc.vector.tensor_mul(t2[:], src_ap(1), cos_hi)
    nc.vector.tensor_mul(t3[:], src_ap(0), sin_hi)
    nc.vector.tensor_tensor(out_ap(1), t2[:], t3[:], mybir.AluOpType.add)


def build_nc(reps=1):
    nc = bacc.Bacc(None, target_bir_lowering=False, debug=False)
    dt_in = {}

    def din(name, shape, dt=BF16):
        t = nc.dram_tensor(name, list(shape), dt, kind="ExternalInput")
        dt_in[name] = t
        return t

    din("xT", (DIM, TL))
    din("cosT_c", (HDR, TL))
    din("sinT_c", (HDR, TL))
    din("WdqT", (DIM, DCQ))
    din("WdkvT", (DIM, DCKV))
    din("WkrT", (DIM, HDR))
    din("WqAT", (DCQ, 2048))    # plain q blocks (replicated), see _prep_inputs
    din("WqRT", (DCQ, 384))     # rope q blocks h10..15 (pre-summed, scaled)
    din("Wk1T", (DCKV, 256))    # k main per slot (per-core)
    din("Wk2aT", (DCKV, 128))   # k ext per slot (per-core)
    din("Wk2bT", (HDR, 128))    # beta * I64 per slot (kr contribution)
    din("WvT", (DCKV, 256))     # v per slot
    din("WoT", (256, DIM))      # W_out cols for my 2 heads, transposed
    outT = nc.dram_tensor("outT", [TL, DIM], BF16, kind="ExternalOutput")

    with tile.TileContext(nc) as tc:
        with tc.tile_pool(name="const", bufs=1) as const, \
             tc.tile_pool(name="dram", bufs=1, space="DRAM") as dram:
            ones = const.tile([128, 1], BF16, tag="ones")
            nc.gpsimd.memset(ones[:], 1.0)
            zeros = const.tile([128, TL], BF16, tag="zeros")
            nc.gpsimd.memset(zeros[:], 0.0)
            masks = []
            for s in range(4):  # keep iff q >= k : y >= p + s*128
                m = const.tile([128, TL], BF16, tag=f"mask{s}")
                nc.gpsimd.memset(m[:], 1.0)
                nc.gpsimd.affine_select(out=m[:], in_=m[:],
                                        compare_op=mybir.AluOpType.is_ge, fill=0.0,
                                        base=-s * 128, pattern=[[1, TL]],
                                        channel_multiplier=-1)
                masks.append(m)

            for _rep in range(reps):
                sfx = f"_{_rep}"
                bounce = dram.tile([KVB, TL], BF16, tag="bounce" + sfx,
                                   name="bounce" + sfx)
                gath = dram.tile([R, KVB, TL], BF16, tag="gath" + sfx,
                                 name="gath" + sfx, addr_space="Shared")
                a2a_in = [dram.tile([R, 192, TL], BF16, tag=f"a2ain{s}{sfx}",
                                    name=f"a2ain{s}{sfx}") for s in range(2)]
                a2a_out = [dram.tile([R, 192, TL], BF16, tag=f"a2aout{s}{sfx}",
                                     name=f"a2aout{s}{sfx}") for s in range(2)]
                rs_in = dram.tile([R, TL, DIM], BF16, tag="rsin" + sfx,
                                  name="rsin" + sfx)
                rs_out = dram.tile([TL, DIM], BF16, tag="rsout" + sfx,
                                   name="rsout" + sfx)
                _phase(nc, tc, ones, zeros, masks,
                       bounce, gath, a2a_in, a2a_out, rs_in, rs_out,
                       dt_in, outT, sfx)

    nc.compile()
    return nc


def _phase(nc, tc, ones, zeros, masks,
           bounce, gath, a2a_in, a2a_out, rs_in, rs_out, dt_in, outT, sfx):
    # ================= Phase A: local down-projections =================
    # cqsb (local c_q, bf16) survives into phase Q.
    with tc.tile_pool(name="pcq" + sfx, bufs=1) as pcq:
        cqsb = pcq.tile([128, NKQ, TL], BF16, tag="cqsb")
        wqa = pcq.tile([128, NKQ, 2048], BF16, tag="wqa")
        wqr = pcq.tile([128, NKQ, 384], BF16, tag="wqr")
        csb = pcq.tile([HDR, TL], BF16, tag="csb")
        ssb = pcq.tile([HDR, TL], BF16, tag="ssb")
        with tc.tile_pool(name="paw" + sfx, bufs=1) as paw, \
             tc.tile_pool(name="pas" + sfx, bufs=3) as pas, \
             tc.tile_pool(name="paps" + sfx, bufs=2, space="PSUM") as paps:
            xsb = paw.tile([128, NKD, TL], BF16, tag="xsb")
            wdkv = paw.tile([128, NKD, DCKV], BF16, tag="wdkv")
            xr = dt_in["xT"].rearrange("(ko p) t -> p ko t", p=128)
            wr = dt_in["WdkvT"].rearrange("(ko p) n -> p ko n", p=128)
            for kc in range(8):
                ksl = slice(2 * kc, 2 * kc + 2)
                nc.sync.dma_start(xsb[:, ksl, :], xr[:, ksl, :])
                nc.sync.dma_start(wdkv[:, ksl, :], wr[:, ksl, :])
            wkr = paw.tile([128, NKD, HDR], BF16, tag="wkr")
            nc.sync.dma_start(wkr[:], dt_in["WkrT"].rearrange("(ko p) n -> p ko n", p=128))
            nc.sync.dma_start(csb[:], dt_in["cosT_c"][:])
            nc.sync.dma_start(ssb[:], dt_in["sinT_c"][:])
            wdq = paw.tile([128, NKD, DCQ], BF16, tag="wdq")
            dq_r = dt_in["WdqT"].rearrange("(ko p) n -> p ko n", p=128)
            with tc.tile_wait_until(0.022):
                for kc in range(4):
                    ksl = slice(4 * kc, 4 * kc + 4)
                    nc.sync.dma_start(wdq[:, ksl, :], dq_r[:, ksl, :])
            nc.sync.dma_start(wqa[:], dt_in["WqAT"].rearrange("(ko p) n -> p ko n", p=128))
            nc.sync.dma_start(wqr[:], dt_in["WqRT"].rearrange("(ko p) n -> p ko n", p=128))

            # c_kv -> bounce rows [0, 512)
            for m in range(NKC):
                ps = paps.tile([128, TL], F32, tag="psA", name="psA")
                for k in range(NKD):
                    nc.tensor.matmul(ps[:], wdkv[:, k, m * 128:(m + 1) * 128],
                                     xsb[:, k, :], start=(k == 0), stop=(k == NKD - 1))
                ev = pas.tile([128, TL], BF16, tag="evA", name="evA")
                nc.scalar.copy(ev[:], ps[:])
                nc.sync.dma_start(bounce[m * 128:(m + 1) * 128, :], ev[:])
            # roped k_r -> bounce rows [512, 576)
            ps = paps.tile([64, TL], F32, tag="pskr")
            for k in range(NKD):
                nc.tensor.matmul(ps[:], wkr[:, k, :], xsb[:, k, :],
                                 start=(k == 0), stop=(k == NKD - 1))
            krr = pas.tile([64, TL], BF16, tag="krr")
            _rope_rows(nc,
                       lambda i: krr[i * 32:(i + 1) * 32, :],
                       lambda i: ps[i * 32:(i + 1) * 32, :],
                       csb[0:32, :], csb[32:64, :], ssb[0:32, :], ssb[32:64, :], pas)
            nc.gpsimd.dma_start(bounce[DCKV:KVB, :], krr[:])
            # c_q -> cqsb (stays on-core)
            for m in range(NKQ):
                ps = paps.tile([128, TL], F32, tag="psA", name="psA")
                for k in range(NKD):
                    nc.tensor.matmul(ps[:], wdq[:, k, m * 128:(m + 1) * 128],
                                     xsb[:, k, :], start=(k == 0), stop=(k == NKD - 1))
                nc.scalar.copy(cqsb[:, m, :], ps[:])

        nc.gpsimd.collective_compute(
            "AllGather", mybir.AluOpType.bypass,
            replica_groups=[list(range(R))],
            ins=[bounce.opt()], outs=[gath.opt()])

        # ============ Phase Q: local q up-projection for all 16 heads ============
        # WqAT col layout (plain blocks, already scaled):
        #   ob 0..7  : slot0 main of head d=ob        -> a2a_in[0][d, 0:128]
        #   ob 8..11 : slot0 ext, heads 2(ob-8)+{0,1} -> a2a_in[0][d, 128:192]
        #   ob 12..14: slot1 main of head 8+(ob-12)   -> a2a_in[1][ob-12, 0:128]
        #   ob 15    : slot1 ext h8 (rows 0:64), h9 (rows 64:128)
        # WqRT: rope blocks rb=0..2 hold heads {10+2rb, 11+2rb} (64 rows each).
        with tc.tile_pool(name="pqs" + sfx, bufs=3) as pqs, \
             tc.tile_pool(name="pqps" + sfx, bufs=2, space="PSUM") as pqps:
            # zero-fill slot1 mains for heads 11..15 (chunks d=3..7)
            for d2 in range(3, 8):
                nc.sync.dma_start(a2a_in[1][d2, 0:128, :], zeros[:])

            for ob in range(16):
                ps = pqps.tile([128, TL], F32, tag="psQ", name="psQ")
                for k in range(NKQ):
                    nc.tensor.matmul(ps[:], wqa[:, k, ob * 128:(ob + 1) * 128],
                                     cqsb[:, k, :], start=(k == 0), stop=(k == NKQ - 1))
                ev = pqs.tile([128, TL], BF16, tag="evQ", name="evQ")
                nc.scalar.copy(ev[:], ps[:])
                if ob < 8:
                    nc.sync.dma_start(a2a_in[0][ob, 0:128, :], ev[:])
                elif ob < 12:
                    for half in range(2):
                        d = 2 * (ob - 8) + half
                        nc.sync.dma_start(a2a_in[0][d, 128:192, :],
                                          ev[half * 64:(half + 1) * 64, :])
                elif ob < 15:
                    nc.sync.dma_start(a2a_in[1][ob - 12, 0:128, :], ev[:])
                else:
                    for half in range(2):
                        nc.sync.dma_start(a2a_in[1][half, 128:192, :],
                                          ev[half * 64:(half + 1) * 64, :])
            for rb in range(3):
                ps = pqps.tile([128, TL], F32, tag="psQ", name="psQ")
                for k in range(NKQ):
                    nc.tensor.matmul(ps[:], wqr[:, k, rb * 128:(rb + 1) * 128],
                                     cqsb[:, k, :], start=(k == 0), stop=(k == NKQ - 1))
                for half in range(2):
                    h = 10 + 2 * rb + half
                    rr = pqs.tile([64, TL], BF16, tag="evQ", name="evQ")
                    off = half * 64
                    _rope_rows(nc,
                               lambda i: rr[i * 32:(i + 1) * 32, :],
                               lambda i, off=off: ps[off + i * 32:off + (i + 1) * 32, :],
                               csb[0:32, :], csb[32:64, :],
                               ssb[0:32, :], ssb[32:64, :], pqs)
                    nc.sync.dma_start(a2a_in[1][h - 8, 128:192, :], rr[:])

    nc.gpsimd.collective_compute(
        "AllToAll", mybir.AluOpType.bypass,
        replica_groups=[list(range(R))],
        ins=[a2a_in[0].opt()], outs=[a2a_out[0].opt()])
    nc.gpsimd.collective_compute(
        "AllToAll", mybir.AluOpType.bypass,
        replica_groups=[list(range(R))],
        ins=[a2a_in[1].opt()], outs=[a2a_out[1].opt()])

    # ============ Phase B: k/v up-projection + attention + out partials ============
    # tile_wait_until: keep the scheduler from hoisting collective-dependent
    # phase-B work ahead of the local q path in the in-order engine queues.
    with tc.tile_wait_until(0.2), \
         tc.tile_pool(name="pbw" + sfx, bufs=1) as pbw, \
         tc.tile_pool(name="pbig" + sfx, bufs=1) as pbig:
        wo = pbw.tile([128, 2, DIM], BF16, tag="wo")
        nc.sync.dma_start(wo[:], dt_in["WoT"].rearrange("(s p) n -> p s n", p=128))

        K1 = pbig.tile([128, 2, R, TL], BF16, tag="K1", name="K1")
        K2 = pbig.tile([128, 2, R, TL], BF16, tag="K2", name="K2")
        V = pbig.tile([128, 32, 256], BF16, tag="V", name="V")
        Q1 = [pbig.tile([128, R, TL], BF16, tag=f"Q1_{s}", name=f"Q1_{s}")
              for s in range(2)]
        Q2 = [pbig.tile([128, R, TL], BF16, tag=f"Q2_{s}", name=f"Q2_{s}")
              for s in range(2)]
        Y = pbig.tile([128, 2, B, 4, TL], BF16, tag="Y", name="Y")

        def warm(pool, tag, n, cols=512):
            for _ in range(n):
                wps = pool.tile([128, cols], F32, tag=tag, name=tag)
                nc.tensor.matmul(wps[:], zeros[:, 0:128], zeros[:], start=True, stop=True)

        with tc.tile_pool(name="pkw" + sfx, bufs=1) as pkw, \
             tc.tile_pool(name="pbc" + sfx, bufs=3) as pbc, \
             tc.tile_pool(name="pbps" + sfx, bufs=2, space="PSUM") as pbps, \
             tc.tile_pool(name="pbp2" + sfx, bufs=2, space="PSUM") as pbp2:
            warm(pbps, "psb", 12)
            wk1 = pkw.tile([128, NKC, 256], BF16, tag="wk1")
            nc.sync.dma_start(wk1[:], dt_in["Wk1T"].rearrange("(ko p) n -> p ko n", p=128))
            wk2a = pkw.tile([128, NKC, 128], BF16, tag="wk2a")
            nc.sync.dma_start(wk2a[:], dt_in["Wk2aT"].rearrange("(ko p) n -> p ko n", p=128))
            wk2b = pkw.tile([128, 128], BF16, tag="wk2b")
            nc.sync.dma_start(wk2b[0:64, :], dt_in["Wk2bT"][:])
            wv = pkw.tile([128, NKC, 256], BF16, tag="wv")
            nc.sync.dma_start(wv[:], dt_in["WvT"].rearrange("(ko p) n -> p ko n", p=128))

            for rt in range(R):
                ckv = pbc.tile([128, NKC, TL], BF16, tag="ckvcol", name="ckvcol")
                nc.gpsimd.dma_start(ckv[:], gath[rt, 0:DCKV, :]
                                    .rearrange("(ko p) t -> p ko t", p=128))
                krg = pbc.tile([128, TL], BF16, tag="krg", name="krg")
                nc.gpsimd.dma_start(krg[0:64, :], gath[rt, DCKV:KVB, :])
                for s in range(2):
                    ps = pbps.tile([128, TL], F32, tag="psb", name="psb")
                    for k in range(NKC):
                        nc.tensor.matmul(ps[:], wk1[:, k, s * 128:(s + 1) * 128],
                                         ckv[:, k, :], start=(k == 0), stop=(k == NKC - 1))
                    nc.scalar.copy(K1[:, s, rt, :], ps[:])
                ps = pbps.tile([128, TL], F32, tag="psb", name="psb")
                for k in range(NKC):
                    nc.tensor.matmul(ps[:], wk2a[:, k, :], ckv[:, k, :],
                                     start=(k == 0), stop=False)
                nc.tensor.matmul(ps[:], wk2b[0:64, :], krg[0:64, :], start=False, stop=True)
                for s in range(2):
                    nc.scalar.copy(K2[0:64, s, rt, :], ps[s * 64:(s + 1) * 64, :])
                for js in range(4):
                    ps = pbp2.tile([128, 256], F32, tag="psv", name="psv")
                    for k in range(NKC):
                        nc.tensor.matmul(ps[:], ckv[:, k, js * 128:(js + 1) * 128],
                                         wv[:, k, :], start=(k == 0), stop=(k == NKC - 1))
                    nc.scalar.copy(V[:, rt * 4 + js, :], ps[:])

        with tc.tile_pool(name="pat" + sfx, bufs=8) as pat, \
             tc.tile_pool(name="pan" + sfx, bufs=4) as pan, \
             tc.tile_pool(name="psS" + sfx, bufs=3, space="PSUM") as psS, \
             tc.tile_pool(name="psY" + sfx, bufs=2, space="PSUM") as psY, \
             tc.tile_pool(name="psL" + sfx, bufs=1, space="PSUM") as psL, \
             tc.tile_pool(name="pcps" + sfx, bufs=2, space="PSUM") as pcps, \
             tc.tile_pool(name="pce" + sfx, bufs=6) as pce:
            for s in range(2):
                with tc.tile_wait_until(0.3 if s == 0 else 0.42):
                    for src in range(R):
                        nc.gpsimd.dma_start(Q1[s][:, src, :], a2a_out[s][src, 0:128, :])
                        nc.gpsimd.dma_start(Q2[s][0:64, src, :], a2a_out[s][src, 128:192, :])

        def do_attn(b, s):
            for it in range(4):
                rti = b * 4 + it
                nj = 4 * (it + 1)
                py = psY.tile([128, TL], F32, tag="py", name="py")
                pl = psL.tile([1, TL], F32, tag="pl", name="pl")
                for j in range(nj):
                    rtj = b * 4 + j // 4
                    sub = j % 4
                    sl = slice(sub * 128, (sub + 1) * 128)
                    pss = psS.tile([128, TL], F32, tag="pss", name="pss")
                    nc.tensor.matmul(pss[:], K1[:, s, rtj, sl], Q1[s][:, rti, :],
                                     start=True, stop=False)
                    nc.tensor.matmul(pss[:], K2[0:64, s, rtj, sl], Q2[s][0:64, rti, :],
                                     start=False, stop=True)
                    et = pat.tile([128, TL], BF16, tag="et", name="et")
                    d = j - 4 * it
                    if d >= 0:
                        er = pat.tile([128, TL], BF16, tag="er", name="er")
                        nc.scalar.activation(er[:], pss[:],
                                             mybir.ActivationFunctionType.Exp)
                        nc.vector.tensor_mul(et[:], er[:], masks[d][:])
                    else:
                        nc.scalar.activation(et[:], pss[:],
                                             mybir.ActivationFunctionType.Exp)
                    jj = b * 16 + j
                    nc.tensor.matmul(py[:], V[:, jj, s * 128:(s + 1) * 128], et[:],
                                     start=(j == 0), stop=(j == nj - 1))
                    nc.tensor.matmul(pl[:], ones[:], et[:],
                                     start=(j == 0), stop=(j == nj - 1))
                rec = pan.tile([1, TL], F32, tag="rec", name="rec")
                nc.vector.reciprocal(rec[:], pl[:])
                rb = pan.tile([128, TL], F32, tag="rb", name="rb")
                nc.gpsimd.partition_broadcast(rb[:], rec[:])
                nc.vector.tensor_mul(Y[:, s, b, it, :], py[:], rb[:])

        def do_out(b):
            # rs_in[b*4+it][tok 512, DIM] partials: out[t, o] = sum_s Y_s^T Wo_s
            for it in range(4):
                for sub in range(4):
                    tsl = slice(sub * 128, (sub + 1) * 128)
                    for obk in range(4):
                        osl = slice(obk * 512, (obk + 1) * 512)
                        po = pcps.tile([128, 512], F32, tag="po", name="po")
                        nc.tensor.matmul(po[:], Y[:, 0, b, it, tsl], wo[:, 0, osl],
                                         start=True, stop=False)
                        nc.tensor.matmul(po[:], Y[:, 1, b, it, tsl], wo[:, 1, osl],
                                         start=False, stop=True)
                        ev = pce.tile([128, 512], BF16, tag="evO", name="evO")
                        if obk % 2 == 0:
                            nc.scalar.copy(ev[:], po[:])
                        else:
                            nc.vector.tensor_copy(ev[:], po[:])
                        nc.sync.dma_start(rs_in[b * 4 + it, tsl, osl], ev[:])

        do_attn(0, 0)
        do_attn(1, 0)
        do_attn(0, 1)
        do_out(0)
        do_attn(1, 1)
        do_out(1)

    nc.gpsimd.collective_compute(
        "ReduceScatter", mybir.AluOpType.add,
        replica_groups=[list(range(R))],
        ins=[rs_in.opt()], outs=[rs_out.opt()])
    nc.sync.dma_start(outT[:], rs_out[:])


def _prep_inputs(inputs):
    """Host-side sharding: returns in_maps list of 8 dicts."""
    x = np.asarray(inputs["x"], np.float32)
    cos = np.asarray(inputs["freq_cos"], np.float32)
    sin = np.asarray(inputs["freq_sin"], np.float32)
    for bn in ("b_dq", "b_uq", "b_qr", "b_dkv", "b_uk", "b_uv", "b_kr", "b_out"):
        assert np.abs(np.asarray(inputs[bn])).max() == 0.0, f"{bn} nonzero"
    W_dq = np.asarray(inputs["W_dq"], np.float32)
    W_uq = np.asarray(inputs["W_uq"], np.float32)
    W_qr = np.asarray(inputs["W_qr"], np.float32)
    W_dkv = np.asarray(inputs["W_dkv"], np.float32)
    W_uk = np.asarray(inputs["W_uk"], np.float32)
    W_uv = np.asarray(inputs["W_uv"], np.float32)
    W_kr = np.asarray(inputs["W_kr"], np.float32)
    W_out = np.asarray(inputs["W_out"], np.float32)

    scale = 1.0 / np.float32(np.sqrt(HD + HDR))
    xf = x.reshape(T, DIM)
    cosT = np.ascontiguousarray(cos.T).astype(BF)   # [64, 2048]
    sinT = np.ascontiguousarray(sin.T).astype(BF)
    WdqT = np.ascontiguousarray(W_dq.T).astype(BF)
    WdkvT = np.ascontiguousarray(W_dkv.T).astype(BF)
    WkrT = np.ascontiguousarray(W_kr.T).astype(BF)

    # replicated q weights: plain blocks [2048, DCQ] + rope blocks [384, DCQ]
    WqA = np.empty((2048, DCQ), np.float32)
    for h in range(8):
        WqA[128 * h:128 * (h + 1)] = W_uq[192 * h:192 * h + 128]
        WqA[1024 + 64 * h:1024 + 64 * (h + 1)] = W_uq[192 * h + 128:192 * h + 192]
    WqA[1536:1664] = W_uq[1536:1664]   # h8 main
    WqA[1664:1792] = W_uq[1728:1856]   # h9 main
    WqA[1792:1920] = W_uq[1920:2048]   # h10 main
    WqA[1920:1984] = W_uq[1664:1728]   # h8 ext
    WqA[1984:2048] = W_uq[1856:1920]   # h9 ext
    WqA *= scale
    WqR = np.empty((384, DCQ), np.float32)
    WqR[0:64] = W_qr[0:64]             # h10 rope
    for h in range(11, 16):
        r0 = 192 * h - 2048
        WqR[64 * (h - 10):64 * (h - 9)] = (W_qr[r0:r0 + 64]
                                           + W_qr[r0 + 64:r0 + 128]
                                           + W_qr[r0 + 128:r0 + 192])
    WqR *= scale
    WqAT = np.ascontiguousarray(WqA.T).astype(BF)
    WqRT = np.ascontiguousarray(WqR.T).astype(BF)

    def k_parts(h):
        km = np.zeros((128, DCKV), np.float32)
        ke = np.zeros((64, DCKV), np.float32)
        beta = 0.0
        if h <= 9:
            km[:] = W_uk[192 * h:192 * h + 128]
            ke[:] = W_uk[192 * h + 128:192 * h + 192]
        elif h == 10:
            km[:] = W_uk[1920:2048]
            beta = 1.0
        else:
            beta = 1.0
        return km, ke, beta

    in_maps = []
    for c in range(R):
        heads = (c, c + 8)
        Wk1 = np.zeros((256, DCKV), np.float32)
        Wk2a = np.zeros((128, DCKV), np.float32)
        Wk2b = np.zeros((128, HDR), np.float32)
        Wv = np.zeros((256, DCKV), np.float32)
        for s, h in enumerate(heads):
            km, ke, beta = k_parts(h)
            Wk1[128 * s:128 * (s + 1)] = km
            Wk2a[64 * s:64 * (s + 1)] = ke
            Wk2b[64 * s:64 * (s + 1)] = beta * np.eye(HDR, dtype=np.float32)
            Wv[128 * s:128 * (s + 1)] = W_uv[128 * h:128 * (h + 1)]
        WoT = np.concatenate(
            [np.ascontiguousarray(W_out[:, 128 * h:128 * (h + 1)].T) for h in heads],
            axis=0)
        p0 = (c % 4) * TL
        in_maps.append({
            "xT": np.ascontiguousarray(xf[c * TL:(c + 1) * TL].T).astype(BF),
            "cosT_c": np.ascontiguousarray(cosT[:, p0:p0 + TL]),
            "sinT_c": np.ascontiguousarray(sinT[:, p0:p0 + TL]),
            "WdqT": WdqT, "WdkvT": WdkvT, "WkrT": WkrT,
            "WqAT": WqAT, "WqRT": WqRT,
            "Wk1T": np.ascontiguousarray(Wk1.T).astype(BF),
            "Wk2aT": np.ascontiguousarray(Wk2a.T).astype(BF),
            "Wk2bT": np.ascontiguousarray(Wk2b.T).astype(BF),
            "WvT": np.ascontiguousarray(Wv.T).astype(BF),
            "WoT": WoT.astype(BF),
        })
    return in_maps


_NC_CACHE = {}


def get_nc(reps=1):
    if reps not in _NC_CACHE:
        _NC_CACHE[reps] = build_nc(reps)
    return _NC_CACHE[reps]


def kernel(**inputs) -> np.ndarray:
    nc = get_nc()
    in_maps = _prep_inputs(inputs)
    res = run_bass_kernel_spmd(nc, in_maps, core_ids=list(range(R)))
    out = np.empty((T, DIM), np.float32)
    for c in range(R):
        out[c * TL:(c + 1) * TL] = res.results[c]["outT"].astype(np.float32)
    return out.reshape(B, S, DIM)
